# revision 35
# baseline (speedup 1.0000x reference)
# kernel.py — prefix-causal multi-head attention block on 8 Trainium2 cores.
#
# Sharding: 32 (batch, head) pairs -> core c owns batch c//4, heads 4*(c%4)..+3.
# Each core: QKV projection for its 4 heads (weights column-sharded), attention
# with block-skipped prefix-causal masking, and a column-sharded partial output
# projection. Host sums the 4 partial projections per batch and adds the bias.
#
# Self-contained: hardcodes shapes (2, 2048, 1024), 16 heads, hd=64.
import sys

for p in ("/opt/trn_rl_repo",):
    if p not in sys.path:
        sys.path.insert(0, p)

import numpy as np
import ml_dtypes

import concourse.bass as bass
import concourse.mybir as mybir
import concourse.tile as tile
from concourse import bacc
from concourse.bass_utils import run_bass_kernel_spmd

BS, S, DIM = 2, 2048, 1024
NHEADS, HD = 16, 64
NCORES = 8
HPC = 4              # heads per core
QG = 512             # q-group (rhs free dim) size
KC = 128             # k-chunk (lhsT free dim) size
NQG = S // QG        # 4
NKC = S // KC        # 16
NEG = -1.0e30

f32 = mybir.dt.float32
f32r = mybir.dt.float32r
bf16 = mybir.dt.bfloat16


def _classify(prefix_mask, num_patches):
    """Per (qg, kc) block class, shared across all cores (union over batches).

    allowed(b, k, q) = (k <= q) or prefix_mask[b, k]   (num_patches >= S assumed)
    Returns dict[(g, kc)] -> 'full' | 'diag' | 'diag_pref' | 'above' | 'skip'.
    """
    pm = np.asarray(prefix_mask, dtype=bool)
    classes = {}
    for g in range(NQG):
        q0 = g * QG
        for kc in range(NKC):
            k0 = kc * KC
            causal_full = (k0 + KC - 1) <= q0
            causal_empty = k0 > q0 + QG - 1
            anypref = bool(pm[:, k0 : k0 + KC].any())
            # fully allowed for every batch?
            full_all = True
            if not causal_full:
                kk = np.arange(k0, k0 + KC)[:, None]
                qq = np.arange(q0, q0 + QG)[None, :]
                causal = kk <= qq
                for b in range(pm.shape[0]):
                    if not np.all(causal | pm[b, k0 : k0 + KC][:, None]):
                        full_all = False
                        break
            if causal_full or full_all:
                cls = "full"
            elif causal_empty:
                cls = "above" if anypref else "skip"
            elif anypref:
                cls = "diag_pref"
            else:
                cls = "diag"
            classes[(g, kc)] = cls
    return classes


def _build_program(classes):
    """Emit the SPMD Tile program (identical for all 8 cores)."""
    nc = bacc.Bacc(None, target_bir_lowering=False, debug=False)

    xT = nc.declare_dram_parameter("xT", [DIM, S], bf16, isOutput=False)
    wqk = nc.declare_dram_parameter("wqk", [DIM, 2 * HPC * HD], bf16, isOutput=False)
    wv = nc.declare_dram_parameter("wv", [DIM, HPC * HD], bf16, isOutput=False)
    pw = nc.declare_dram_parameter("pw", [HPC * HD, DIM], bf16, isOutput=False)
    pref = nc.declare_dram_parameter("pref", [KC, NKC], f32, isOutput=False)
    yT = nc.declare_dram_parameter("yT", [DIM, S], bf16, isOutput=True)

    need_dpref = any(c == "diag_pref" for c in classes.values())

    DC = DIM // 128  # 8 contraction chunks

    with tile.TileContext(nc) as tc:
        with (
            tc.tile_pool(name="const", bufs=1) as const,
            tc.tile_pool(name="xpool", bufs=DC) as xpool,
            tc.tile_pool(name="wpool", bufs=1) as wpool,
            tc.tile_pool(name="qkpool", bufs=4) as qkpool,
            tc.tile_pool(name="ctxpool", bufs=2) as ctxpool,
            tc.tile_pool(name="exppool", bufs=6) as exppool,
            tc.tile_pool(name="smallpool", bufs=8) as smallpool,
            tc.tile_pool(name="outpool", bufs=3) as outpool,
            tc.tile_pool(name="ps_big", bufs=2, space="PSUM") as ps_big,
            tc.tile_pool(name="ps_s", bufs=2, space="PSUM") as ps_s,
            tc.tile_pool(name="ps_pv", bufs=2, space="PSUM") as ps_pv,
        ):
            # ---- constants ----
            tri = const.tile([KC, KC], f32)  # tri[p, j] = p <= j ? 0 : NEG
            nc.gpsimd.memset(tri[:], 0.0)
            # keep 0 where j >= p (iota = j - p >= 0), else NEG
            nc.gpsimd.affine_select(
                out=tri[:], in_=tri[:], compare_op=mybir.AluOpType.is_ge,
                fill=NEG, base=0, pattern=[[1, KC]], channel_multiplier=-1,
            )
            caus_full = {}
            if need_dpref:
                for r in (0, 128, 256, 384):
                    cf = const.tile([KC, QG], f32, tag=f"cf{r}")
                    nc.gpsimd.memset(cf[:], 0.0)
                    # keep 0 where fg - p - r >= 0 (k0+p <= q0+fg)
                    nc.gpsimd.affine_select(
                        out=cf[:], in_=cf[:], compare_op=mybir.AluOpType.is_ge,
                        fill=NEG, base=-r, pattern=[[1, QG]], channel_multiplier=-1,
                    )
                    caus_full[r] = cf

            pref_sb = const.tile([KC, NKC], f32)
            nc.sync.dma_start(out=pref_sb[:], in_=pref[:])

            # vext[:, ti, l, 0:64] = V rows, col 64 = 1.0 (sums column)
            vext = const.tile([128, NKC, HPC, HD + 1], bf16)
            nc.gpsimd.memset(vext[:, :, :, HD : HD + 1], 1.0)

            # ---- load weights FIRST (first matmul needs wqk), then x ----
            wqk_sb = wpool.tile([128, DC, 2 * HPC * HD], bf16)
            nc.sync.dma_start(
                out=wqk_sb[:], in_=wqk.rearrange("(do di) o -> di do o", di=128)
            )
            wv_sb = wpool.tile([128, DC, HPC * HD], bf16)
            nc.scalar.dma_start(
                out=wv_sb[:], in_=wv.rearrange("(do di) o -> di do o", di=128)
            )
            x_sb = [
                xpool.tile([128, S], bf16, tag="x", name=f"x{dc}") for dc in range(DC)
            ]
            xT3 = xT.rearrange("(do di) t -> di do t", di=128)
            for dc in range(DC):
                eng = nc.sync if dc % 2 == 0 else nc.scalar
                eng.dma_start(out=x_sb[dc][:, 0 : S // 2], in_=xT3[:, dc, 0 : S // 2])
                eng.dma_start(
                    out=x_sb[dc][:, S // 2 : S], in_=xT3[:, dc, S // 2 : S]
                )

            pw_sb = wpool.tile([128, 2, DIM], bf16)
            nc.scalar.dma_start(
                out=pw_sb[:], in_=pw.rearrange("(vo vi) u -> vi vo u", vi=128)
            )

            # ---- v projection, emitted in ranges interleaved with pair 0 ----
            def emit_v(lo=0, hi=NKC):
              for ti in range(lo, hi):
                ps = ps_big.tile([128, HPC * HD], f32, tag="big")
                for dc in range(DC):
                    nc.tensor.matmul(
                        ps[:],
                        x_sb[dc][:, ti * 128 : (ti + 1) * 128],
                        wv_sb[:, dc, :],
                        start=(dc == 0), stop=(dc == DC - 1),
                    )
                nc.vector.tensor_copy(
                    vext[:, ti, :, 0:HD],
                    ps.rearrange("p (l d) -> p l d", l=HPC),
                )

            # ---- q/k projection + attention, interleaved per head pair ----
            # qk_sb[oi] holds sel-rows [oi*128, oi*128+128) over all tokens.
            # oi 0..1 = q rows (pre-scaled), oi 2..3 = k rows.
            qk_sb = [
                qkpool.tile([128, S], f32r, tag="qk", name=f"qk{i}") for i in range(4)
            ]
            ctxT_sb = [
                ctxpool.tile([128, S], bf16, tag="ctx", name=f"ctx{i}")
                for i in range(2)
            ]

            _qkn = [0]

            def emit_qk_proj(oi):
                for tc_i in range(S // 512):
                    gi = _qkn[0]
                    _qkn[0] += 1
                    if gi < 8 and gi % 3 == 1:
                        pss = ps_s.tile([128, 2, QG], f32, tag="s", name="qkrs")
                        ps = pss[:, 0, :]
                    elif gi < 8 and gi % 3 == 2:
                        psp = ps_pv.tile([128, QG], f32, tag="pv", name="qkrp")
                        ps = psp[:, :]
                    else:
                        ps = ps_big.tile([128, 512], f32, tag="big", name="qkps")
                    for dc in range(DC):
                        nc.tensor.matmul(
                            ps[:],
                            wqk_sb[:, dc, oi * 128 : (oi + 1) * 128],
                            x_sb[dc][:, tc_i * 512 : (tc_i + 1) * 512],
                            start=(dc == 0), stop=(dc == DC - 1),
                        )
                    nc.vector.tensor_copy(
                        qk_sb[oi][:, tc_i * 512 : (tc_i + 1) * 512], ps[:]
                    )

            def emit_pair(a, after_g=None, before_g=None):
                # heads la = 2a (partitions 0:64 of qk tiles) and lb = 2a+1
                # (partitions 64:128). One [128, 2, QG] score tile holds the
                # same (g, kc) block for BOTH heads; exp + diag masking fuse.
                la, lb = 2 * a, 2 * a + 1
                qi, ki = a, 2 + a
                for g in range(NQG):
                    if before_g is not None:
                        before_g(g)
                    blocks = [
                        (kc, classes[(g, kc)])
                        for kc in range(NKC)
                        if classes[(g, kc)] != "skip"
                    ]
                    pv0 = ps_pv.tile([HD + 1, QG], f32, tag="pv", name="pv0")
                    pv1 = ps_pv.tile([HD + 1, QG], f32, tag="pv", name="pv1")
                    nblk = len(blocks)
                    for bi, (kc, cls) in enumerate(blocks):
                        spg = ps_s.tile([128, 2, QG], f32, tag="s")
                        ex = exppool.tile([128, 2, QG], bf16, tag="exp")
                        for j, po in ((0, 0), (1, 64)):
                            nc.tensor.matmul(
                                spg[:, j, :],
                                qk_sb[ki][po : po + 64, kc * KC : (kc + 1) * KC],
                                qk_sb[qi][po : po + 64, g * QG : (g + 1) * QG],
                                start=True, stop=True,
                            )
                        r = kc * KC - g * QG
                        if cls == "diag_pref":
                            for j in range(2):
                                tmp = smallpool.tile([KC, QG], f32, tag="dpref")
                                nc.vector.tensor_max(
                                    tmp[:], caus_full[r][:],
                                    pref_sb[:, kc : kc + 1].to_broadcast((KC, QG)),
                                )
                                nc.vector.tensor_add(
                                    spg[:, j, :], spg[:, j, :], tmp[:]
                                )
                        if cls == "above":
                            nc.scalar.activation(
                                ex[:, :, :], spg[:, :, :],
                                mybir.ActivationFunctionType.Exp,
                                bias=pref_sb[:, kc : kc + 1],
                            )
                        elif cls == "diag" and r > 0:
                            nc.scalar.activation(
                                ex[:, :, r:QG], spg[:, :, r:QG],
                                mybir.ActivationFunctionType.Exp,
                            )
                        else:
                            nc.scalar.activation(
                                ex[:, :, :], spg[:, :, :],
                                mybir.ActivationFunctionType.Exp,
                            )
                        if cls == "diag":
                            if r > 0:
                                nc.gpsimd.memset(ex[:, :, 0:r], 0.0)
                            # zero strictly-above-diagonal in both heads at once
                            nc.gpsimd.affine_select(
                                out=ex[:, :, r : r + KC],
                                in_=ex[:, :, r : r + KC],
                                compare_op=mybir.AluOpType.is_ge,
                                fill=0.0, base=0,
                                pattern=[[0, 2], [1, KC]], channel_multiplier=-1,
                            )
                        st, sp = bi == 0, bi == nblk - 1
                        nc.tensor.matmul(
                            pv0[:], vext[:, kc, la, :], ex[:, 0, :],
                            start=st, stop=sp,
                        )
                        nc.tensor.matmul(
                            pv1[:], vext[:, kc, lb, :], ex[:, 1, :],
                            start=st, stop=sp,
                        )
                    # normalize: ctxT = pv[0:64] * (1 / pv[64]) broadcast.
                    # Copy psum -> sbuf first so the pv bank frees early.
                    for po, pv in ((0, pv0), (64, pv1)):
                        pvs = smallpool.tile([HD + 1, QG], f32, tag="pvs")
                        nc.vector.tensor_copy(pvs[:], pv[:])
                        rcp = smallpool.tile([1, QG], f32, tag="rcp")
                        nc.vector.reciprocal(rcp[:], pvs[HD : HD + 1, :])
                        bcast = smallpool.tile([64, QG], f32, tag="bcast")
                        nc.gpsimd.partition_broadcast(bcast[:], rcp[:])
                        nc.vector.tensor_mul(
                            ctxT_sb[a][po : po + 64, g * QG : (g + 1) * QG],
                            pvs[0:HD, :],
                            bcast[:],
                        )
                    if after_g is not None:
                        after_g(g)


            # ---- output projection: emitted per q-group inside pair 1 ----
            yT3 = yT.rearrange("(uo ui) t -> ui uo t", ui=128)
            dma_engs = [nc.sync, nc.scalar]

            def emit_proj_g(tc_i):
                for ui in range(DIM // 128):
                    ps = ps_big.tile([128, QG], f32, tag="big")
                    for vo in range(2):
                        nc.tensor.matmul(
                            ps[:],
                            pw_sb[:, vo, ui * 128 : (ui + 1) * 128],
                            ctxT_sb[vo][:, tc_i * QG : (tc_i + 1) * QG],
                            start=(vo == 0), stop=(vo == 1),
                        )
                    ot = outpool.tile([128, QG], bf16, tag="out")
                    nc.vector.tensor_copy(ot[:], ps[:])
                    dma_engs[ui % len(dma_engs)].dma_start(
                        out=yT3[:, ui, tc_i * QG : (tc_i + 1) * QG], in_=ot[:]
                    )

            emit_qk_proj(2)
            emit_qk_proj(0)

            emit_v()
            emit_pair(0)
            emit_qk_proj(3)
            emit_qk_proj(1)
            emit_pair(1, after_g=emit_proj_g)

    nc.compile()
    return nc


_cache = {}


def _get_program(classes):
    key = tuple(sorted(classes.items()))
    if key not in _cache:
        _cache[key] = _build_program(classes)
    return _cache[key]


def _make_in_maps(x, qkv_w, proj_w, prefix_mask):
    scale = float(HD) ** -0.5
    pm = np.asarray(prefix_mask, dtype=bool)
    in_maps = []
    for c in range(NCORES):
        b, hg = c // 4, c % 4
        r0 = hg * HPC * HD  # 256-row slice start inside each q/k/v block
        wq = qkv_w[r0 : r0 + HPC * HD] * scale
        wk = qkv_w[DIM + r0 : DIM + r0 + HPC * HD]
        wv_ = qkv_w[2 * DIM + r0 : 2 * DIM + r0 + HPC * HD]
        pref_add = np.where(pm[b], 0.0, NEG).astype(np.float32)
        in_maps.append({
            "xT": np.ascontiguousarray(x[b].T).astype(ml_dtypes.bfloat16),
            "wqk": np.ascontiguousarray(np.concatenate([wq, wk], 0).T).astype(ml_dtypes.bfloat16),
            "wv": np.ascontiguousarray(wv_.T).astype(ml_dtypes.bfloat16),
            "pw": np.ascontiguousarray(
                proj_w[:, r0 : r0 + HPC * HD].T
            ).astype(ml_dtypes.bfloat16),
            "pref": np.ascontiguousarray(pref_add.reshape(NKC, KC).T),
            "yT": np.zeros((DIM, S), ml_dtypes.bfloat16),
        })
    return in_maps


last_exec_time_ns = None


def kernel(x, qkv_w, proj_w, proj_b, prefix_mask, num_patches, trace=False):
    global last_exec_time_ns
    x = np.asarray(x, np.float32)
    qkv_w = np.asarray(qkv_w, np.float32)
    proj_w = np.asarray(proj_w, np.float32)
    proj_b = np.asarray(proj_b, np.float32)
    assert int(num_patches) >= S
    classes = _classify(prefix_mask, num_patches)
    nc = _get_program(classes)
    in_maps = _make_in_maps(x, qkv_w, proj_w, prefix_mask)
    res = run_bass_kernel_spmd(nc, in_maps, list(range(NCORES)), trace=trace)
    last_exec_time_ns = res.exec_time_ns
    out = np.empty((BS, S, DIM), np.float32)
    for b in range(BS):
        acc = res.results[4 * b]["yT"].astype(np.float32)
        for c in range(4 * b + 1, 4 * b + 4):
            acc += res.results[c]["yT"].astype(np.float32)
        out[b] = acc.T + proj_b
    return out
